# revision 11
# baseline (speedup 1.0000x reference)
"""Trainium2 Bass kernel for nn_CycleEmbedding0 (gnn_message_passing).

Computes out = segment_sum(emb_W[x][atom_to_cycle[0]], atom_to_cycle[1], 200000).

Key algebraic reduction: the embedding table has only VOCAB=22 rows, so
    out[c, :] = sum_v H[c, v] * emb_W[v, :]
where H[c, v] = #{pairs p : seg[p] == c and x[src[p]] == v} is a class
histogram.  This cuts memory traffic ~8x vs the naive gather/scatter.

Distribution (8 NeuronCores): cycle bins are range-sharded across cores
(25000 bins/core).  On the host, each core's bins are load-balanced into
392 windows of 64 bin-slots (serpentine assignment by bin popularity) and
the core's pairs are bucketed per window, padded to C chunks of 128.

Device kernel per core (identical SPMD program):
  stage 1 (histogram): per 256-pair double-chunk, TensorE accumulates
    psum_HT[v, slot] += sum_i OC[:,i,:].T @ OH[:,i,:] with fp8 DoubleRow
    matmuls (2 MACs/cell/cycle).  OH (slot one-hots) and OC (class
    one-hots) are built on the host as fp8 and streamed in.
  stage 2 (apply emb): per window-pair, out[128, 128] = HT^T @ (W_hi+W_lo)
    with emb_W split into two bf16 matrices for fp32-level accuracy;
    ScalarE evacuates the histogram, VectorE evacuates the output;
    DMA writes [25088, 128] f32 per core.

Host gathers the 8 core outputs and un-permutes rows back to cycle order.
"""

import numpy as np
import ml_dtypes
from contextlib import ExitStack

import concourse.bass as bass
import concourse.tile as tile
import concourse.mybir as mybir
from concourse import bacc
from concourse.bass_utils import run_bass_kernel_spmd

BF16 = ml_dtypes.bfloat16
FP8 = ml_dtypes.float8_e4m3

N_ATOMS = 500000
N_PAIRS = 2000000
N_CYCLES = 200000
VOCAB = 22
HIDDEN = 128

NCORES = 8
BPC = N_CYCLES // NCORES      # bins (cycles) per core
W = 48                        # bin-slots per window
PW = 2 * W                    # rows per window-pair
VC = 32                       # class cols padded (DoubleRow needs step%16==0)
NWIN = 528                    # windows per core
NBLK = 24                     # OH/OC streamed in NBLK blocks
WPB = NWIN // NBLK            # windows per block
GROUP = 4                     # windows per psum group
SMAX = -(-BPC // NWIN)        # max slot index + 1 (<= 64)
assert SMAX <= W and NWIN % GROUP == 0 and NWIN % NBLK == 0

_prog_cache: dict = {}


def _build_program(C: int):
    """One SPMD program; C (even) chunks of 128 pairs per window."""
    assert C % 2 == 0
    D = C // 2                    # double-chunks per window
    NCH2 = NWIN * D               # double-chunks per core
    nc = bacc.Bacc("TRN2", target_bir_lowering=False, debug=False,
                   num_devices=NCORES)
    wmat_d = nc.dram_tensor("wmat", [VOCAB, 2 * HIDDEN], mybir.dt.bfloat16,
                            kind="ExternalInput")
    oh_d = nc.dram_tensor("oh", [128, NCH2 * 2 * W], mybir.dt.float8e4,
                          kind="ExternalInput")
    oc_d = nc.dram_tensor("oc", [128, NCH2 * 2 * VC], mybir.dt.float8e4,
                          kind="ExternalInput")
    out_d = nc.dram_tensor("out", [NWIN * W, HIDDEN], mybir.dt.float32,
                           kind="ExternalOutput")
    out_ap = out_d.ap()
    ohcols = WPB * D * 2 * W      # per-block free dim
    occols = WPB * D * 2 * VC

    with tile.TileContext(nc) as tc:
        with ExitStack() as ctx:
            const = ctx.enter_context(tc.tile_pool(name="const", bufs=1))
            ohpool = ctx.enter_context(tc.tile_pool(name="ohblk", bufs=4))
            ocpool = ctx.enter_context(tc.tile_pool(name="ocblk", bufs=4))
            htpool = ctx.enter_context(tc.tile_pool(name="hts", bufs=3))
            outpool = ctx.enter_context(tc.tile_pool(name="outs", bufs=3))
            ps_ht = ctx.enter_context(
                tc.tile_pool(name="psht", bufs=3, space=bass.MemorySpace.PSUM))
            ps_out = ctx.enter_context(
                tc.tile_pool(name="psout", bufs=2, space=bass.MemorySpace.PSUM))

            wmat = const.tile([VOCAB, 2 * HIDDEN], mybir.dt.bfloat16)
            nc.default_dma_engine.dma_start(wmat[:], wmat_d.ap())

            oh_t: dict = {}
            oc_t: dict = {}

            def load_block(blk):
                t = ohpool.tile([128, ohcols], mybir.dt.float8e4)
                oh_eng = nc.scalar if blk % 6 == 5 else nc.sync
                oh_eng.dma_start(
                    t[:], oh_d.ap()[:, blk * ohcols:(blk + 1) * ohcols])
                oh_t[blk] = t
                t = ocpool.tile([128, occols], mybir.dt.float8e4)
                nc.gpsimd.dma_start(
                    t[:], oc_d.ap()[:, blk * occols:(blk + 1) * occols])
                oc_t[blk] = t

            for blk in range(min(3, NBLK)):
                load_block(blk)

            outs_box = [None]

            def stage2(g, ht):
                hts = htpool.tile([VOCAB, GROUP * W], mybir.dt.bfloat16)
                nc.scalar.copy(hts[:], ht[0:VOCAB, :])
                ops = ps_out.tile([PW, 2 * HIDDEN], mybir.dt.float32)
                for wp in range(2):
                    lhsT = hts[:, wp * PW:(wp + 1) * PW]
                    o = ops[:, wp * HIDDEN:(wp + 1) * HIDDEN]
                    nc.tensor.matmul(o, lhsT, wmat[:, 0:HIDDEN],
                                     start=True, stop=False)
                    nc.tensor.matmul(o, lhsT, wmat[:, HIDDEN:2 * HIDDEN],
                                     start=False, stop=True)
                # batch 2 groups per SBUF tile so the out-DMA moves 2 KiB
                # per partition row instead of 512 B
                half = g % 2
                if half == 0:
                    outs_box[0] = outpool.tile([PW, 4 * HIDDEN],
                                               mybir.dt.float32, name="outs", tag="outs")
                outs = outs_box[0]
                nc.vector.tensor_copy(
                    outs[:, half * 2 * HIDDEN:(half + 1) * 2 * HIDDEN], ops[:])
                if half == 1:
                    g0 = g - 1
                    dst = out_ap[g0 * GROUP * W:(g0 + 2) * GROUP * W,
                                 :].rearrange("(wp b) h -> b wp h", wp=4)
                    nc.scalar.dma_start(dst, outs[:].rearrange(
                        "b (wp h) -> b wp h", wp=4))

            pending = None
            for g in range(NWIN // GROUP):
                ht = ps_ht.tile([VC, GROUP * W], mybir.dt.float32)
                for wi in range(GROUP):
                    w = g * GROUP + wi
                    blk, wloc = divmod(w, WPB)
                    if wloc == 0 and blk + 3 < NBLK:
                        load_block(blk + 3)
                    for dc in range(D):
                        j = wloc * D + dc
                        oh3 = oh_t[blk][:, j * 2 * W:(j + 1) * 2 * W].rearrange(
                            "p (two s) -> p two s", two=2)
                        oc3 = oc_t[blk][:, j * 2 * VC:(j + 1) * 2 * VC].rearrange(
                            "p (two v) -> p two v", two=2)
                        nc.tensor.matmul(
                            ht[:, wi * W:(wi + 1) * W], oc3, oh3,
                            start=(dc == 0), stop=(dc == D - 1),
                            perf_mode=mybir.MatmulPerfMode.DoubleRow)
                if pending is not None:
                    stage2(*pending)
                pending = (g, ht)
            stage2(*pending)
    nc.compile()
    return nc


_EYE_OH = np.zeros((W + 1, W), FP8)
_EYE_OH[np.arange(W), np.arange(W)] = 1
_EYE_OC = np.zeros((VOCAB + 1, VC), FP8)
_EYE_OC[np.arange(VOCAB), np.arange(VOCAB)] = 1


def _prep_core(local: np.ndarray, cls: np.ndarray, C: int | None):
    """Window-balance one core's pairs.  Returns wcnt_max when C is None,
    else (oh[128, NCH2*128] fp8, oc[128, NCH2*64] fp8, row_of_local[BPC])."""
    cnt = np.bincount(local, minlength=BPC)
    order = np.argsort(cnt, kind="stable")[::-1]
    r = np.arange(BPC)
    passi, pos = divmod(r, NWIN)
    wser = np.where(passi % 2 == 0, pos, NWIN - 1 - pos)
    w_of_bin = np.empty(BPC, np.int32)
    s_of_bin = np.empty(BPC, np.int32)
    w_of_bin[order] = wser
    s_of_bin[order] = passi
    wkey = w_of_bin[local]
    wcnt = np.bincount(wkey, minlength=NWIN)
    if C is None:
        return int(wcnt.max())

    NCH2 = NWIN * C // 2
    order1 = np.argsort(wkey, kind="stable")
    wsorted = wkey[order1]
    starts = np.zeros(NWIN, np.int64)
    np.cumsum(wcnt[:-1], out=starts[1:])
    idx_in_w = np.arange(len(local)) - starts[wsorted]
    dest = wsorted.astype(np.int64) * (C * 128) + idx_in_w

    slot_pad = np.full(NWIN * C * 128, W, np.int16)
    slot_pad[dest] = s_of_bin[local[order1]]
    cls_pad = np.full(NWIN * C * 128, VOCAB, np.int16)
    cls_pad[dest] = cls[order1]

    oh_in = np.ascontiguousarray(
        _EYE_OH[slot_pad].reshape(NCH2, 2, 128, W).transpose(2, 0, 1, 3)
    ).reshape(128, NCH2 * 2 * W)
    oc_in = np.ascontiguousarray(
        _EYE_OC[cls_pad].reshape(NCH2, 2, 128, VC).transpose(2, 0, 1, 3)
    ).reshape(128, NCH2 * 2 * VC)
    row_of_local = (w_of_bin * W + s_of_bin).astype(np.int64)
    return oh_in, oc_in, row_of_local


def _make_in_maps(x, atom_to_cycle, emb_W, C=None):
    src = np.asarray(atom_to_cycle[0], dtype=np.int64)
    seg = np.asarray(atom_to_cycle[1], dtype=np.int64)
    cls_all = np.asarray(x, dtype=np.int16)[src]

    order0 = np.argsort(seg, kind="stable")
    seg_s = seg[order0]
    cls_s = cls_all[order0]
    bounds = np.searchsorted(seg_s, np.arange(NCORES + 1) * BPC)

    cores = []
    for c in range(NCORES):
        lo, hi = bounds[c], bounds[c + 1]
        cores.append((np.asarray(seg_s[lo:hi] - c * BPC, np.int64),
                      cls_s[lo:hi]))

    if C is None:
        wmax = max(_prep_core(l, k, None) for l, k in cores)
        C = max(4, 2 * (-(-wmax // 256)))

    w32 = np.asarray(emb_W, np.float32)
    w_hi = w32.astype(BF16)
    w_lo = (w32 - w_hi.astype(np.float32)).astype(BF16)
    wmat_in = np.concatenate([w_hi, w_lo], axis=1)

    in_maps, rowmaps = [], []
    for local, k in cores:
        oh_in, oc_in, rowmap = _prep_core(local, k, C)
        in_maps.append({"wmat": wmat_in, "oh": oh_in, "oc": oc_in})
        rowmaps.append(rowmap)
    return C, in_maps, rowmaps


def kernel(x, atom_to_cycle, emb_W, n_cycles):
    assert int(n_cycles) == N_CYCLES
    x = np.asarray(x)
    atom_to_cycle = np.asarray(atom_to_cycle)
    emb_W = np.asarray(emb_W, np.float32)
    assert atom_to_cycle.shape == (2, N_PAIRS) and emb_W.shape == (VOCAB, HIDDEN)

    C, in_maps, rowmaps = _make_in_maps(x, atom_to_cycle, emb_W)
    if C not in _prog_cache:
        _prog_cache[C] = _build_program(C)
    nc = _prog_cache[C]

    res = run_bass_kernel_spmd(nc, in_maps, list(range(NCORES))).results

    out = np.empty((N_CYCLES, HIDDEN), np.float32)
    for c in range(NCORES):
        out[c * BPC:(c + 1) * BPC] = res[c]["out"][rowmaps[c]]
    return out


# revision 16
# speedup vs baseline: 1.2032x; 1.2032x over previous
"""Trainium2 Bass kernel for nn_CycleEmbedding0 (gnn_message_passing).

Computes out = segment_sum(emb_W[x][atom_to_cycle[0]], atom_to_cycle[1], 200000).

Key algebraic reduction: the embedding table has only VOCAB=22 rows, so
    out[c, :] = sum_v H[c, v] * emb_W[v, :]
where H[c, v] = #{pairs p : seg[p] == c and x[src[p]] == v} is a class
histogram.  This cuts memory traffic ~8x vs the naive gather/scatter.

Distribution (8 NeuronCores): cycle bins are range-sharded across cores
(25000 bins/core).  On the host, each core's bins are load-balanced into
392 windows of 64 bin-slots (serpentine assignment by bin popularity) and
the core's pairs are bucketed per window, padded to C chunks of 128.

Device kernel per core (identical SPMD program):
  stage 1 (histogram): per 256-pair double-chunk, TensorE accumulates
    psum_HT[v, slot] += sum_i OC[:,i,:].T @ OH[:,i,:] with fp8 DoubleRow
    matmuls (2 MACs/cell/cycle).  OH (slot one-hots) and OC (class
    one-hots) are built on the host as fp8 and streamed in.
  stage 2 (apply emb): per window-pair, out[128, 128] = HT^T @ (W_hi+W_lo)
    with emb_W split into two bf16 matrices for fp32-level accuracy;
    ScalarE evacuates the histogram, VectorE evacuates the output;
    DMA writes [25088, 128] f32 per core.

Host gathers the 8 core outputs and un-permutes rows back to cycle order.
"""

import numpy as np
import ml_dtypes
from contextlib import ExitStack

import concourse.bass as bass
import concourse.tile as tile
import concourse.mybir as mybir
from concourse import bacc
from concourse.bass_utils import run_bass_kernel_spmd

BF16 = ml_dtypes.bfloat16
FP8 = ml_dtypes.float8_e4m3

N_ATOMS = 500000
N_PAIRS = 2000000
N_CYCLES = 200000
VOCAB = 22
HIDDEN = 128

NCORES = 8
BPC = N_CYCLES // NCORES      # bins (cycles) per core
W = 64                        # bin-slots per window
PW = 2 * W                    # rows per window-pair
VC = 32                       # class cols padded (DoubleRow needs step%16==0)
NWIN = 392                    # windows per core
NBLK = 28                     # OH/OC streamed in NBLK blocks
WPB = NWIN // NBLK            # windows per block
GROUP = 4                     # windows per psum group
SMAX = -(-BPC // NWIN)        # max slot index + 1 (<= 64)
assert SMAX <= W and NWIN % GROUP == 0 and NWIN % NBLK == 0

_prog_cache: dict = {}


def _build_program(C: int):
    """One SPMD program; C (even) chunks of 128 pairs per window."""
    assert C % 2 == 0
    D = C // 2                    # double-chunks per window
    NCH2 = NWIN * D               # double-chunks per core
    nc = bacc.Bacc("TRN2", target_bir_lowering=False, debug=False,
                   num_devices=NCORES)
    wmat_d = nc.dram_tensor("wmat", [VOCAB, 2 * HIDDEN], mybir.dt.bfloat16,
                            kind="ExternalInput")
    oh_d = nc.dram_tensor("oh", [128, NCH2 * 2 * W], mybir.dt.float8e4,
                          kind="ExternalInput")
    oc_d = nc.dram_tensor("oc", [128, NCH2 * 2 * VC], mybir.dt.float8e4,
                          kind="ExternalInput")
    out_d = nc.dram_tensor("out", [NWIN * W, HIDDEN], mybir.dt.float32,
                           kind="ExternalOutput")
    out_ap = out_d.ap()
    ohcols = WPB * D * 2 * W      # per-block free dim
    occols = WPB * D * 2 * VC

    with tile.TileContext(nc) as tc:
        with ExitStack() as ctx:
            const = ctx.enter_context(tc.tile_pool(name="const", bufs=1))
            ohpool = ctx.enter_context(tc.tile_pool(name="ohblk", bufs=4))
            ocpool = ctx.enter_context(tc.tile_pool(name="ocblk", bufs=4))
            htpool = ctx.enter_context(tc.tile_pool(name="hts", bufs=3))
            outpool = ctx.enter_context(tc.tile_pool(name="outs", bufs=3))
            ps_ht = ctx.enter_context(
                tc.tile_pool(name="psht", bufs=3, space=bass.MemorySpace.PSUM))
            ps_out = ctx.enter_context(
                tc.tile_pool(name="psout", bufs=2, space=bass.MemorySpace.PSUM))

            wmat = const.tile([VOCAB, 2 * HIDDEN], mybir.dt.bfloat16)
            nc.default_dma_engine.dma_start(wmat[:], wmat_d.ap())

            oh_t: dict = {}
            oc_t: dict = {}

            def load_block(blk):
                t = ohpool.tile([128, ohcols], mybir.dt.float8e4)
                oh_eng = nc.scalar if blk % 6 == 5 else nc.sync
                oh_eng.dma_start(
                    t[:], oh_d.ap()[:, blk * ohcols:(blk + 1) * ohcols])
                oh_t[blk] = t
                t = ocpool.tile([128, occols], mybir.dt.float8e4)
                nc.gpsimd.dma_start(
                    t[:], oc_d.ap()[:, blk * occols:(blk + 1) * occols])
                oc_t[blk] = t

            for blk in range(min(3, NBLK)):
                load_block(blk)

            outs_box = [None]

            def stage2(g, ht):
                hts = htpool.tile([VOCAB, GROUP * W], mybir.dt.bfloat16)
                nc.scalar.copy(hts[:], ht[0:VOCAB, :])
                ops = ps_out.tile([PW, 2 * HIDDEN], mybir.dt.float32)
                for wp in range(2):
                    lhsT = hts[:, wp * PW:(wp + 1) * PW]
                    o = ops[:, wp * HIDDEN:(wp + 1) * HIDDEN]
                    nc.tensor.matmul(o, lhsT, wmat[:, 0:HIDDEN],
                                     start=True, stop=False)
                    nc.tensor.matmul(o, lhsT, wmat[:, HIDDEN:2 * HIDDEN],
                                     start=False, stop=True)
                # batch 4 groups per SBUF tile so the out-DMA moves 4 KiB
                # per partition row; fuse the hi+lo add into the evacuation
                half = g % 4
                if half == 0:
                    outs_box[0] = outpool.tile([PW, 8 * HIDDEN],
                                               mybir.dt.float32, name="outs", tag="outs")
                outs = outs_box[0]
                nc.vector.tensor_copy(
                    outs[:, half * 2 * HIDDEN:(half + 1) * 2 * HIDDEN], ops[:])
                if half == 3 or g == NWIN // GROUP - 1:
                    nb = half + 1
                    g0 = g - half
                    dst = out_ap[g0 * GROUP * W:(g0 + nb) * GROUP * W,
                                 :].rearrange("(wp b) h -> b wp h", wp=2 * nb)
                    nc.scalar.dma_start(
                        dst, outs[:, :nb * 2 * HIDDEN].rearrange(
                            "b (wp h) -> b wp h", wp=2 * nb))

            pending = None
            for g in range(NWIN // GROUP):
                ht = ps_ht.tile([VC, GROUP * W], mybir.dt.float32)
                for wi in range(GROUP):
                    w = g * GROUP + wi
                    blk, wloc = divmod(w, WPB)
                    if wloc == 0 and blk + 3 < NBLK:
                        load_block(blk + 3)
                    for dc in range(D):
                        j = wloc * D + dc
                        oh3 = oh_t[blk][:, j * 2 * W:(j + 1) * 2 * W].rearrange(
                            "p (two s) -> p two s", two=2)
                        oc3 = oc_t[blk][:, j * 2 * VC:(j + 1) * 2 * VC].rearrange(
                            "p (two v) -> p two v", two=2)
                        nc.tensor.matmul(
                            ht[:, wi * W:(wi + 1) * W], oc3, oh3,
                            start=(dc == 0), stop=(dc == D - 1),
                            perf_mode=mybir.MatmulPerfMode.DoubleRow)
                if pending is not None:
                    stage2(*pending)
                pending = (g, ht)
            stage2(*pending)
    nc.compile()
    return nc


_EYE_OH = np.zeros((W + 1, W), FP8)
_EYE_OH[np.arange(W), np.arange(W)] = 1
_EYE_OC = np.zeros((VOCAB + 1, VC), FP8)
_EYE_OC[np.arange(VOCAB), np.arange(VOCAB)] = 1


def _prep_core(local: np.ndarray, cls: np.ndarray, C: int | None):
    """Window-balance one core's pairs.  Returns wcnt_max when C is None,
    else (oh[128, NCH2*128] fp8, oc[128, NCH2*64] fp8, row_of_local[BPC])."""
    cnt = np.bincount(local, minlength=BPC)
    order = np.argsort(cnt, kind="stable")[::-1]
    r = np.arange(BPC)
    passi, pos = divmod(r, NWIN)
    wser = np.where(passi % 2 == 0, pos, NWIN - 1 - pos)
    w_of_bin = np.empty(BPC, np.int32)
    s_of_bin = np.empty(BPC, np.int32)
    w_of_bin[order] = wser
    s_of_bin[order] = passi
    wkey = w_of_bin[local]
    wcnt = np.bincount(wkey, minlength=NWIN)
    if C is None:
        return int(wcnt.max())

    NCH2 = NWIN * C // 2
    order1 = np.argsort(wkey, kind="stable")
    wsorted = wkey[order1]
    starts = np.zeros(NWIN, np.int64)
    np.cumsum(wcnt[:-1], out=starts[1:])
    idx_in_w = np.arange(len(local)) - starts[wsorted]
    dest = wsorted.astype(np.int64) * (C * 128) + idx_in_w

    slot_pad = np.full(NWIN * C * 128, W, np.int16)
    slot_pad[dest] = s_of_bin[local[order1]]
    cls_pad = np.full(NWIN * C * 128, VOCAB, np.int16)
    cls_pad[dest] = cls[order1]

    oh_in = np.ascontiguousarray(
        _EYE_OH[slot_pad].reshape(NCH2, 2, 128, W).transpose(2, 0, 1, 3)
    ).reshape(128, NCH2 * 2 * W)
    oc_in = np.ascontiguousarray(
        _EYE_OC[cls_pad].reshape(NCH2, 2, 128, VC).transpose(2, 0, 1, 3)
    ).reshape(128, NCH2 * 2 * VC)
    row_of_local = (w_of_bin * W + s_of_bin).astype(np.int64)
    return oh_in, oc_in, row_of_local


def _make_in_maps(x, atom_to_cycle, emb_W, C=None):
    src = np.asarray(atom_to_cycle[0], dtype=np.int64)
    seg = np.asarray(atom_to_cycle[1], dtype=np.int64)
    cls_all = np.asarray(x, dtype=np.int16)[src]

    order0 = np.argsort(seg, kind="stable")
    seg_s = seg[order0]
    cls_s = cls_all[order0]
    bounds = np.searchsorted(seg_s, np.arange(NCORES + 1) * BPC)

    cores = []
    for c in range(NCORES):
        lo, hi = bounds[c], bounds[c + 1]
        cores.append((np.asarray(seg_s[lo:hi] - c * BPC, np.int64),
                      cls_s[lo:hi]))

    if C is None:
        wmax = max(_prep_core(l, k, None) for l, k in cores)
        C = max(6, 2 * (-(-wmax // 256)))

    w32 = np.asarray(emb_W, np.float32)
    w_hi = w32.astype(BF16)
    w_lo = (w32 - w_hi.astype(np.float32)).astype(BF16)
    wmat_in = np.concatenate([w_hi, w_lo], axis=1)

    in_maps, rowmaps = [], []
    for local, k in cores:
        oh_in, oc_in, rowmap = _prep_core(local, k, C)
        in_maps.append({"wmat": wmat_in, "oh": oh_in, "oc": oc_in})
        rowmaps.append(rowmap)
    return C, in_maps, rowmaps


def kernel(x, atom_to_cycle, emb_W, n_cycles):
    assert int(n_cycles) == N_CYCLES
    x = np.asarray(x)
    atom_to_cycle = np.asarray(atom_to_cycle)
    emb_W = np.asarray(emb_W, np.float32)
    assert atom_to_cycle.shape == (2, N_PAIRS) and emb_W.shape == (VOCAB, HIDDEN)

    C, in_maps, rowmaps = _make_in_maps(x, atom_to_cycle, emb_W)
    if C not in _prog_cache:
        _prog_cache[C] = _build_program(C)
    nc = _prog_cache[C]

    res = run_bass_kernel_spmd(nc, in_maps, list(range(NCORES))).results

    out = np.empty((N_CYCLES, HIDDEN), np.float32)
    for c in range(NCORES):
        out[c * BPC:(c + 1) * BPC] = res[c]["out"][rowmaps[c]]
    return out


# revision 17
# speedup vs baseline: 1.3058x; 1.0853x over previous
"""Trainium2 Bass kernel for nn_CycleEmbedding0 (gnn_message_passing).

Computes out = segment_sum(emb_W[x][atom_to_cycle[0]], atom_to_cycle[1], 200000).

Key algebraic reduction: the embedding table has only VOCAB=22 rows, so
    out[c, :] = sum_v H[c, v] * emb_W[v, :]
where H[c, v] = #{pairs p : seg[p] == c and x[src[p]] == v} is a class
histogram.  This cuts memory traffic ~8x vs the naive gather/scatter.

Distribution (8 NeuronCores): cycle bins are range-sharded across cores
(25000 bins/core).  On the host, each core's bins are packed into 392
windows of 64 bin-slots using a two-tier serpentine (heavy bins fill
tier-A windows with 3 double-chunks of capacity, light bins fill tier-B
windows with 2), and the core's pairs are bucketed per window.

Device kernel per core (identical SPMD program):
  stage 1 (histogram): per 256-pair double-chunk, TensorE accumulates
    psum_HT[v, slot] += sum_i OC[:,i,:].T @ OH[:,i,:] with fp8 DoubleRow
    matmuls (2 MACs/cell/cycle).  OH (slot one-hots) and OC (class
    one-hots) are built on the host as fp8 and streamed in on separate
    DMA queues.
  stage 2 (apply emb, software-pipelined one group behind stage 1):
    per window-pair, out[128, 128] = HT^T @ W_hi + HT^T @ W_lo with
    emb_W split into two bf16 matrices for fp32-level accuracy; ScalarE
    evacuates the histogram, VectorE the output (batched 4 groups so the
    out-DMA moves 4 KiB per partition row).

Host gathers the 8 core outputs and un-permutes rows back to cycle order.
"""

import numpy as np
import ml_dtypes
from contextlib import ExitStack

import concourse.bass as bass
import concourse.tile as tile
import concourse.mybir as mybir
from concourse import bacc
from concourse.bass_utils import run_bass_kernel_spmd

BF16 = ml_dtypes.bfloat16
FP8 = ml_dtypes.float8_e4m3

N_ATOMS = 500000
N_PAIRS = 2000000
N_CYCLES = 200000
VOCAB = 22
HIDDEN = 128

NCORES = 8
BPC = N_CYCLES // NCORES      # bins (cycles) per core
W = 64                        # bin-slots per window
PW = 2 * W                    # rows per window-pair
VC = 32                       # class cols padded (DoubleRow needs step%16==0)
NWIN = 392                    # windows per core
NBLK = 28                     # OH/OC streamed in NBLK blocks
WPB = NWIN // NBLK            # windows per block
GROUP = 4                     # windows per psum group
assert NWIN % GROUP == 0 and NWIN % NBLK == 0

# Candidate per-window double-chunk templates, tried in order.  dw must be
# non-increasing and tier boundaries must be GROUP-aligned.
_TEMPLATES = [
    (3,) * 284 + (2,) * 108,
    (3,) * NWIN,
    (4,) * NWIN,
    (6,) * NWIN,
    (8,) * NWIN,
    (16,) * NWIN,
]

_prog_cache: dict = {}


def _woff2(dw):
    off = np.zeros(NWIN + 1, np.int64)
    np.cumsum(dw, out=off[1:])
    return off


def _build_program(dw):
    """One SPMD program; dw[w] = double-chunks (256-pair units) of window w."""
    woff2 = _woff2(dw)
    NCH2 = int(woff2[-1])
    nc = bacc.Bacc("TRN2", target_bir_lowering=False, debug=False,
                   num_devices=NCORES)
    wmat_d = nc.dram_tensor("wmat", [VOCAB, 2 * HIDDEN], mybir.dt.bfloat16,
                            kind="ExternalInput")
    oh_d = nc.dram_tensor("oh", [128, NCH2 * 2 * W], mybir.dt.float8e4,
                          kind="ExternalInput")
    oc_d = nc.dram_tensor("oc", [128, NCH2 * 2 * VC], mybir.dt.float8e4,
                          kind="ExternalInput")
    out_d = nc.dram_tensor("out", [NWIN * W, HIDDEN], mybir.dt.float32,
                           kind="ExternalOutput")
    out_ap = out_d.ap()

    with tile.TileContext(nc) as tc:
        with ExitStack() as ctx:
            const = ctx.enter_context(tc.tile_pool(name="const", bufs=1))
            ohpool = ctx.enter_context(tc.tile_pool(name="ohblk", bufs=4))
            ocpool = ctx.enter_context(tc.tile_pool(name="ocblk", bufs=4))
            htpool = ctx.enter_context(tc.tile_pool(name="hts", bufs=3))
            outpool = ctx.enter_context(tc.tile_pool(name="outs", bufs=3))
            ps_ht = ctx.enter_context(
                tc.tile_pool(name="psht", bufs=3, space=bass.MemorySpace.PSUM))
            ps_out = ctx.enter_context(
                tc.tile_pool(name="psout", bufs=2, space=bass.MemorySpace.PSUM))

            wmat = const.tile([VOCAB, 2 * HIDDEN], mybir.dt.bfloat16)
            nc.default_dma_engine.dma_start(wmat[:], wmat_d.ap())

            oh_t: dict = {}
            oc_t: dict = {}

            def load_block(blk):
                j0, j1 = int(woff2[blk * WPB]), int(woff2[(blk + 1) * WPB])
                t = ohpool.tile([128, (j1 - j0) * 2 * W], mybir.dt.float8e4,
                                name="ohb", tag="ohb")
                oh_eng = nc.scalar if blk % 6 == 5 else nc.sync
                oh_eng.dma_start(
                    t[:], oh_d.ap()[:, j0 * 2 * W:j1 * 2 * W])
                oh_t[blk] = (t, j0)
                t = ocpool.tile([128, (j1 - j0) * 2 * VC], mybir.dt.float8e4,
                                name="ocb", tag="ocb")
                nc.gpsimd.dma_start(
                    t[:], oc_d.ap()[:, j0 * 2 * VC:j1 * 2 * VC])
                oc_t[blk] = (t, j0)

            for blk in range(min(3, NBLK)):
                load_block(blk)

            outs_box = [None]

            def stage2(g, ht):
                hts = htpool.tile([VOCAB, GROUP * W], mybir.dt.bfloat16)
                nc.scalar.copy(hts[:], ht[0:VOCAB, :])
                ops = ps_out.tile([PW, 2 * HIDDEN], mybir.dt.float32)
                for wp in range(2):
                    lhsT = hts[:, wp * PW:(wp + 1) * PW]
                    o = ops[:, wp * HIDDEN:(wp + 1) * HIDDEN]
                    nc.tensor.matmul(o, lhsT, wmat[:, 0:HIDDEN],
                                     start=True, stop=False)
                    nc.tensor.matmul(o, lhsT, wmat[:, HIDDEN:2 * HIDDEN],
                                     start=False, stop=True)
                # batch 4 groups per SBUF tile so the out-DMA moves 4 KiB
                # per partition row
                half = g % 4
                if half == 0:
                    outs_box[0] = outpool.tile(
                        [PW, 8 * HIDDEN], mybir.dt.float32,
                        name="outs", tag="outs")
                outs = outs_box[0]
                nc.vector.tensor_copy(
                    outs[:, half * 2 * HIDDEN:(half + 1) * 2 * HIDDEN], ops[:])
                if half == 3 or g == NWIN // GROUP - 1:
                    nb = half + 1
                    g0 = g - half
                    dst = out_ap[g0 * GROUP * W:(g0 + nb) * GROUP * W,
                                 :].rearrange("(wp b) h -> b wp h", wp=2 * nb)
                    nc.scalar.dma_start(
                        dst, outs[:, :nb * 2 * HIDDEN].rearrange(
                            "b (wp h) -> b wp h", wp=2 * nb))

            pending = None
            for g in range(NWIN // GROUP):
                ht = ps_ht.tile([VC, GROUP * W], mybir.dt.float32)
                for wi in range(GROUP):
                    w = g * GROUP + wi
                    blk, wloc = divmod(w, WPB)
                    if wloc == 0 and blk + 3 < NBLK:
                        load_block(blk + 3)
                    oht, oj0 = oh_t[blk]
                    oct_, cj0 = oc_t[blk]
                    D = dw[w]
                    for dc in range(D):
                        j = int(woff2[w]) + dc - oj0
                        oh3 = oht[:, j * 2 * W:(j + 1) * 2 * W].rearrange(
                            "p (two s) -> p two s", two=2)
                        oc3 = oct_[:, j * 2 * VC:(j + 1) * 2 * VC].rearrange(
                            "p (two v) -> p two v", two=2)
                        nc.tensor.matmul(
                            ht[:, wi * W:(wi + 1) * W], oc3, oh3,
                            start=(dc == 0), stop=(dc == D - 1),
                            perf_mode=mybir.MatmulPerfMode.DoubleRow)
                if pending is not None:
                    stage2(*pending)
                pending = (g, ht)
            stage2(*pending)
    nc.compile()
    return nc


_EYE_OH = np.zeros((W + 1, W), FP8)
_EYE_OH[np.arange(W), np.arange(W)] = 1
_EYE_OC = np.zeros((VOCAB + 1, VC), FP8)
_EYE_OC[np.arange(VOCAB), np.arange(VOCAB)] = 1


def _assign(cnt, dw):
    """Tiered serpentine: heaviest bins to the highest-capacity windows.
    Returns (w_of_bin, s_of_bin)."""
    order = np.argsort(cnt, kind="stable")[::-1]
    w_of_bin = np.empty(BPC, np.int32)
    s_of_bin = np.empty(BPC, np.int32)
    pos0 = 0
    w0 = 0
    while w0 < NWIN and pos0 < BPC:
        w1 = w0
        while w1 < NWIN and dw[w1] == dw[w0]:
            w1 += 1
        nw = w1 - w0
        nb = min(nw * W, BPC - pos0)
        idx = order[pos0:pos0 + nb]
        r = np.arange(nb)
        passi, pos = divmod(r, nw)
        wser = np.where(passi % 2 == 0, pos, nw - 1 - pos) + w0
        w_of_bin[idx] = wser
        s_of_bin[idx] = passi
        pos0 += nb
        w0 = w1
    return w_of_bin, s_of_bin


def _pack_core(local, cls, dw, check_only=False):
    """Bucket one core's pairs per window.  Returns None if some window
    overflows its dw[w]*256 pair capacity; else (oh, oc, row_of_local)."""
    cnt = np.bincount(local, minlength=BPC)
    w_of_bin, s_of_bin = _assign(cnt, dw)
    wkey = w_of_bin[local]
    wcnt = np.bincount(wkey, minlength=NWIN)
    caps = np.asarray(dw, np.int64) * 256
    if (wcnt > caps).any():
        return None
    if check_only:
        return True

    woff2 = _woff2(dw)
    NCH2 = int(woff2[-1])
    order1 = np.argsort(wkey, kind="stable")
    wsorted = wkey[order1]
    starts = np.zeros(NWIN, np.int64)
    np.cumsum(wcnt[:-1], out=starts[1:])
    idx_in_w = np.arange(len(local)) - starts[wsorted]
    dest = woff2[wsorted] * 256 + idx_in_w

    slot_pad = np.full(NCH2 * 256, W, np.int16)
    slot_pad[dest] = s_of_bin[local[order1]]
    cls_pad = np.full(NCH2 * 256, VOCAB, np.int16)
    cls_pad[dest] = cls[order1]

    oh_in = np.ascontiguousarray(
        _EYE_OH[slot_pad].reshape(NCH2, 2, 128, W).transpose(2, 0, 1, 3)
    ).reshape(128, NCH2 * 2 * W)
    oc_in = np.ascontiguousarray(
        _EYE_OC[cls_pad].reshape(NCH2, 2, 128, VC).transpose(2, 0, 1, 3)
    ).reshape(128, NCH2 * 2 * VC)
    row_of_local = (w_of_bin * W + s_of_bin).astype(np.int64)
    return oh_in, oc_in, row_of_local


def _make_in_maps(x, atom_to_cycle, emb_W):
    src = np.asarray(atom_to_cycle[0], dtype=np.int64)
    seg = np.asarray(atom_to_cycle[1], dtype=np.int64)
    cls_all = np.asarray(x, dtype=np.int16)[src]

    order0 = np.argsort(seg, kind="stable")
    seg_s = seg[order0]
    cls_s = cls_all[order0]
    bounds = np.searchsorted(seg_s, np.arange(NCORES + 1) * BPC)

    cores = []
    for c in range(NCORES):
        lo, hi = bounds[c], bounds[c + 1]
        cores.append((np.asarray(seg_s[lo:hi] - c * BPC, np.int64),
                      cls_s[lo:hi]))

    dw = None
    for cand in _TEMPLATES:
        if all(_pack_core(l, k, cand, check_only=True) for l, k in cores):
            dw = cand
            break
    assert dw is not None, "no feasible window template"

    w32 = np.asarray(emb_W, np.float32)
    w_hi = w32.astype(BF16)
    w_lo = (w32 - w_hi.astype(np.float32)).astype(BF16)
    wmat_in = np.concatenate([w_hi, w_lo], axis=1)

    in_maps, rowmaps = [], []
    for local, k in cores:
        oh_in, oc_in, rowmap = _pack_core(local, k, dw)
        in_maps.append({"wmat": wmat_in, "oh": oh_in, "oc": oc_in})
        rowmaps.append(rowmap)
    return dw, in_maps, rowmaps


def kernel(x, atom_to_cycle, emb_W, n_cycles):
    assert int(n_cycles) == N_CYCLES
    x = np.asarray(x)
    atom_to_cycle = np.asarray(atom_to_cycle)
    emb_W = np.asarray(emb_W, np.float32)
    assert atom_to_cycle.shape == (2, N_PAIRS) and emb_W.shape == (VOCAB, HIDDEN)

    dw, in_maps, rowmaps = _make_in_maps(x, atom_to_cycle, emb_W)
    if dw not in _prog_cache:
        _prog_cache[dw] = _build_program(dw)
    nc = _prog_cache[dw]

    res = run_bass_kernel_spmd(nc, in_maps, list(range(NCORES))).results

    out = np.empty((N_CYCLES, HIDDEN), np.float32)
    for c in range(NCORES):
        out[c * BPC:(c + 1) * BPC] = res[c]["out"][rowmaps[c]]
    return out


# revision 19
# speedup vs baseline: 1.3211x; 1.0118x over previous
"""Trainium2 Bass kernel for nn_CycleEmbedding0 (gnn_message_passing).

Computes out = segment_sum(emb_W[x][atom_to_cycle[0]], atom_to_cycle[1], 200000).

Key algebraic reduction: the embedding table has only VOCAB=22 rows, so
    out[c, :] = sum_v H[c, v] * emb_W[v, :]
where H[c, v] = #{pairs p : seg[p] == c and x[src[p]] == v} is a class
histogram.  This cuts memory traffic ~8x vs the naive gather/scatter.

Distribution (8 NeuronCores): cycle bins are range-sharded across cores
(25000 bins/core).  On the host, each core's bins are packed into 392
windows of 64 bin-slots using a two-tier serpentine (heavy bins fill
tier-A windows with 3 double-chunks of capacity, light bins fill tier-B
windows with 2), and the core's pairs are bucketed per window.

Device kernel per core (identical SPMD program):
  stage 1 (histogram): per 256-pair double-chunk, TensorE accumulates
    psum_HT[v, slot] += sum_i OC[:,i,:].T @ OH[:,i,:] with fp8 DoubleRow
    matmuls (2 MACs/cell/cycle).  OH (slot one-hots) and OC (class
    one-hots) are built on the host as fp8 and streamed in on separate
    DMA queues.
  stage 2 (apply emb, software-pipelined one group behind stage 1):
    per window-pair, out[128, 128] = HT^T @ W_hi + HT^T @ W_lo with
    emb_W split into two bf16 matrices for fp32-level accuracy; ScalarE
    evacuates the histogram, VectorE the output (batched 4 groups so the
    out-DMA moves 4 KiB per partition row).

Host gathers the 8 core outputs and un-permutes rows back to cycle order.
"""

import numpy as np
import ml_dtypes
from contextlib import ExitStack

import concourse.bass as bass
import concourse.tile as tile
import concourse.mybir as mybir
from concourse import bacc
from concourse.bass_utils import run_bass_kernel_spmd

BF16 = ml_dtypes.bfloat16
FP8 = ml_dtypes.float8_e4m3

N_ATOMS = 500000
N_PAIRS = 2000000
N_CYCLES = 200000
VOCAB = 22
HIDDEN = 128

NCORES = 8
BPC = N_CYCLES // NCORES      # bins (cycles) per core
W = 64                        # bin-slots per window
PW = 2 * W                    # rows per window-pair
VC = 32                       # class cols padded (DoubleRow needs step%16==0)
NWIN = 392                    # windows per core
NBLK = 56                     # OH/OC streamed in NBLK blocks
WPB = NWIN // NBLK            # windows per block
GROUP = 4                     # windows per psum group
assert NWIN % GROUP == 0 and NWIN % NBLK == 0

# Candidate per-window double-chunk templates, tried in order.  dw must be
# non-increasing and tier boundaries must be GROUP-aligned.
_TEMPLATES = [
    (3,) * 284 + (2,) * 108,
    (3,) * NWIN,
    (4,) * NWIN,
    (6,) * NWIN,
    (8,) * NWIN,
    (16,) * NWIN,
]

_prog_cache: dict = {}


def _woff2(dw):
    off = np.zeros(NWIN + 1, np.int64)
    np.cumsum(dw, out=off[1:])
    return off


def _build_program(dw):
    """One SPMD program; dw[w] = double-chunks (256-pair units) of window w."""
    woff2 = _woff2(dw)
    NCH2 = int(woff2[-1])
    nc = bacc.Bacc("TRN2", target_bir_lowering=False, debug=False,
                   num_devices=NCORES)
    wmat_d = nc.dram_tensor("wmat", [VC + VOCAB, HIDDEN], mybir.dt.bfloat16,
                            kind="ExternalInput")
    oh_d = nc.dram_tensor("oh", [128, NCH2 * 2 * W], mybir.dt.float8e4,
                          kind="ExternalInput")
    oc_d = nc.dram_tensor("oc", [128, NCH2 * 2 * VC], mybir.dt.float8e4,
                          kind="ExternalInput")
    out_d = nc.dram_tensor("out", [NWIN * W, HIDDEN], mybir.dt.float32,
                           kind="ExternalOutput")
    out_ap = out_d.ap()

    with tile.TileContext(nc) as tc:
        with ExitStack() as ctx:
            const = ctx.enter_context(tc.tile_pool(name="const", bufs=1))
            ohpool = ctx.enter_context(tc.tile_pool(name="ohblk", bufs=6))
            ocpool = ctx.enter_context(tc.tile_pool(name="ocblk", bufs=6))
            htpool = ctx.enter_context(tc.tile_pool(name="hts", bufs=3))
            outpool = ctx.enter_context(tc.tile_pool(name="outs", bufs=3))
            ps_ht = ctx.enter_context(
                tc.tile_pool(name="psht", bufs=3, space=bass.MemorySpace.PSUM))
            ps_out = ctx.enter_context(
                tc.tile_pool(name="psout", bufs=2, space=bass.MemorySpace.PSUM))

            wmat = const.tile([VC + VOCAB, HIDDEN], mybir.dt.bfloat16)
            nc.default_dma_engine.dma_start(wmat[:], wmat_d.ap())

            oh_t: dict = {}
            oc_t: dict = {}

            def load_block(blk):
                j0, j1 = int(woff2[blk * WPB]), int(woff2[(blk + 1) * WPB])
                t = ohpool.tile([128, (j1 - j0) * 2 * W], mybir.dt.float8e4,
                                name="ohb", tag="ohb")
                oh_eng = nc.scalar if blk % 6 == 5 else nc.sync
                oh_eng.dma_start(
                    t[:], oh_d.ap()[:, j0 * 2 * W:j1 * 2 * W])
                oh_t[blk] = (t, j0)
                t = ocpool.tile([128, (j1 - j0) * 2 * VC], mybir.dt.float8e4,
                                name="ocb", tag="ocb")
                nc.gpsimd.dma_start(
                    t[:], oc_d.ap()[:, j0 * 2 * VC:j1 * 2 * VC])
                oc_t[blk] = (t, j0)

            for blk in range(min(5, NBLK)):
                load_block(blk)

            outs_box = [None]

            def stage2(g, ht):
                # hts on partitions [0:32] (rows 22:32 are exact zeros)
                # and replicated on [32:54]; one K=54 matmul against
                # [W_hi; 0; W_lo] does hi+lo in one pass
                hts = htpool.tile([VC + VOCAB, GROUP * W], mybir.dt.bfloat16)
                nc.scalar.copy(hts[0:VC, :], ht[:])
                nc.scalar.copy(hts[VC:VC + VOCAB, :], ht[0:VOCAB, :])
                ops = ps_out.tile([PW, 2 * HIDDEN], mybir.dt.float32)
                for wp in range(2):
                    lhsT = hts[:, wp * PW:(wp + 1) * PW]
                    o = ops[:, wp * HIDDEN:(wp + 1) * HIDDEN]
                    nc.tensor.matmul(o, lhsT, wmat[:], start=True, stop=True)
                # batch 4 groups per SBUF tile so the out-DMA moves 4 KiB
                # per partition row
                half = g % 4
                if half == 0:
                    outs_box[0] = outpool.tile(
                        [PW, 8 * HIDDEN], mybir.dt.float32,
                        name="outs", tag="outs")
                outs = outs_box[0]
                nc.vector.tensor_copy(
                    outs[:, half * 2 * HIDDEN:(half + 1) * 2 * HIDDEN], ops[:])
                if half == 3 or g == NWIN // GROUP - 1:
                    nb = half + 1
                    g0 = g - half
                    dst = out_ap[g0 * GROUP * W:(g0 + nb) * GROUP * W,
                                 :].rearrange("(wp b) h -> b wp h", wp=2 * nb)
                    nc.scalar.dma_start(
                        dst, outs[:, :nb * 2 * HIDDEN].rearrange(
                            "b (wp h) -> b wp h", wp=2 * nb))

            pending = None
            for g in range(NWIN // GROUP):
                ht = ps_ht.tile([VC, GROUP * W], mybir.dt.float32)
                for wi in range(GROUP):
                    w = g * GROUP + wi
                    blk, wloc = divmod(w, WPB)
                    if wloc == 0 and blk + 5 < NBLK:
                        load_block(blk + 5)
                    oht, oj0 = oh_t[blk]
                    oct_, cj0 = oc_t[blk]
                    D = dw[w]
                    for dc in range(D):
                        j = int(woff2[w]) + dc - oj0
                        oh3 = oht[:, j * 2 * W:(j + 1) * 2 * W].rearrange(
                            "p (two s) -> p two s", two=2)
                        oc3 = oct_[:, j * 2 * VC:(j + 1) * 2 * VC].rearrange(
                            "p (two v) -> p two v", two=2)
                        nc.tensor.matmul(
                            ht[:, wi * W:(wi + 1) * W], oc3, oh3,
                            start=(dc == 0), stop=(dc == D - 1),
                            perf_mode=mybir.MatmulPerfMode.DoubleRow)
                if pending is not None:
                    stage2(*pending)
                pending = (g, ht)
            stage2(*pending)
    nc.compile()
    return nc


_EYE_OH = np.zeros((W + 1, W), FP8)
_EYE_OH[np.arange(W), np.arange(W)] = 1
_EYE_OC = np.zeros((VOCAB + 1, VC), FP8)
_EYE_OC[np.arange(VOCAB), np.arange(VOCAB)] = 1


def _assign(cnt, dw):
    """Tiered serpentine: heaviest bins to the highest-capacity windows.
    Returns (w_of_bin, s_of_bin)."""
    order = np.argsort(cnt, kind="stable")[::-1]
    w_of_bin = np.empty(BPC, np.int32)
    s_of_bin = np.empty(BPC, np.int32)
    pos0 = 0
    w0 = 0
    while w0 < NWIN and pos0 < BPC:
        w1 = w0
        while w1 < NWIN and dw[w1] == dw[w0]:
            w1 += 1
        nw = w1 - w0
        nb = min(nw * W, BPC - pos0)
        idx = order[pos0:pos0 + nb]
        r = np.arange(nb)
        passi, pos = divmod(r, nw)
        wser = np.where(passi % 2 == 0, pos, nw - 1 - pos) + w0
        w_of_bin[idx] = wser
        s_of_bin[idx] = passi
        pos0 += nb
        w0 = w1
    return w_of_bin, s_of_bin


def _pack_core(local, cls, dw, check_only=False):
    """Bucket one core's pairs per window.  Returns None if some window
    overflows its dw[w]*256 pair capacity; else (oh, oc, row_of_local)."""
    cnt = np.bincount(local, minlength=BPC)
    w_of_bin, s_of_bin = _assign(cnt, dw)
    wkey = w_of_bin[local]
    wcnt = np.bincount(wkey, minlength=NWIN)
    caps = np.asarray(dw, np.int64) * 256
    if (wcnt > caps).any():
        return None
    if check_only:
        return True

    woff2 = _woff2(dw)
    NCH2 = int(woff2[-1])
    order1 = np.argsort(wkey, kind="stable")
    wsorted = wkey[order1]
    starts = np.zeros(NWIN, np.int64)
    np.cumsum(wcnt[:-1], out=starts[1:])
    idx_in_w = np.arange(len(local)) - starts[wsorted]
    dest = woff2[wsorted] * 256 + idx_in_w

    slot_pad = np.full(NCH2 * 256, W, np.int16)
    slot_pad[dest] = s_of_bin[local[order1]]
    cls_pad = np.full(NCH2 * 256, VOCAB, np.int16)
    cls_pad[dest] = cls[order1]

    oh_in = np.ascontiguousarray(
        _EYE_OH[slot_pad].reshape(NCH2, 2, 128, W).transpose(2, 0, 1, 3)
    ).reshape(128, NCH2 * 2 * W)
    oc_in = np.ascontiguousarray(
        _EYE_OC[cls_pad].reshape(NCH2, 2, 128, VC).transpose(2, 0, 1, 3)
    ).reshape(128, NCH2 * 2 * VC)
    row_of_local = (w_of_bin * W + s_of_bin).astype(np.int64)
    return oh_in, oc_in, row_of_local


def _make_in_maps(x, atom_to_cycle, emb_W):
    src = np.asarray(atom_to_cycle[0], dtype=np.int64)
    seg = np.asarray(atom_to_cycle[1], dtype=np.int64)
    cls_all = np.asarray(x, dtype=np.int16)[src]

    order0 = np.argsort(seg, kind="stable")
    seg_s = seg[order0]
    cls_s = cls_all[order0]
    bounds = np.searchsorted(seg_s, np.arange(NCORES + 1) * BPC)

    cores = []
    for c in range(NCORES):
        lo, hi = bounds[c], bounds[c + 1]
        cores.append((np.asarray(seg_s[lo:hi] - c * BPC, np.int64),
                      cls_s[lo:hi]))

    dw = None
    for cand in _TEMPLATES:
        if all(_pack_core(l, k, cand, check_only=True) for l, k in cores):
            dw = cand
            break
    assert dw is not None, "no feasible window template"

    w32 = np.asarray(emb_W, np.float32)
    w_hi = w32.astype(BF16)
    w_lo = (w32 - w_hi.astype(np.float32)).astype(BF16)
    wmat_in = np.concatenate(
        [w_hi, np.zeros((VC - VOCAB, HIDDEN), BF16), w_lo], axis=0)

    in_maps, rowmaps = [], []
    for local, k in cores:
        oh_in, oc_in, rowmap = _pack_core(local, k, dw)
        in_maps.append({"wmat": wmat_in, "oh": oh_in, "oc": oc_in})
        rowmaps.append(rowmap)
    return dw, in_maps, rowmaps


def kernel(x, atom_to_cycle, emb_W, n_cycles):
    assert int(n_cycles) == N_CYCLES
    x = np.asarray(x)
    atom_to_cycle = np.asarray(atom_to_cycle)
    emb_W = np.asarray(emb_W, np.float32)
    assert atom_to_cycle.shape == (2, N_PAIRS) and emb_W.shape == (VOCAB, HIDDEN)

    dw, in_maps, rowmaps = _make_in_maps(x, atom_to_cycle, emb_W)
    if dw not in _prog_cache:
        _prog_cache[dw] = _build_program(dw)
    nc = _prog_cache[dw]

    res = run_bass_kernel_spmd(nc, in_maps, list(range(NCORES))).results

    out = np.empty((N_CYCLES, HIDDEN), np.float32)
    for c in range(NCORES):
        out[c * BPC:(c + 1) * BPC] = res[c]["out"][rowmaps[c]]
    return out
